# revision 31
# baseline (speedup 1.0000x reference)
"""Trainium2 Bass kernel for nn_AAConv (attention-augmented conv block).

Sharding: data-parallel over batch B=8 across the 8 NeuronCores (one sample
per core).  Each core computes, for its sample x [256, 32, 32]:

  conv_out = conv3x3(x, conv_w) + conv_b              -> channels 0..255
  kqv      = 1x1conv(x, attn_w) + attn_b
  k, q, v  = split(kqv);  q *= dkh^-0.5
  attn     = softmax(q^T k per head) @ v              -> channels 256..511
  out      = concat(conv_out, attn_out)               [512, 32, 32]

Key implementation choices (hardware-profile driven):
  * all matmuls in bf16 (fp32 is 4 cyc/row on the PE), fp32 PSUM accumulate.
  * attention in transposed-logits layout: logitsT[m, n] = k^T q per head,
    exp'd directly PSUM->SBUF on ScalarE (no max subtraction needed:
    logits ~ N(0,1)).  The softmax denominator rides along as a 33rd
    ones-column of the attn@v lhsT, then gets partition-redistributed with a
    tiny broadcast DMA for the final divide.
  * one flat software pipeline over all 64 (head-pair, n-tile, m-tile)
    steps: the logits of step s are scheduled before the attn consumers of
    step s-2 so the exp stream on ScalarE (the critical engine) never waits
    behind unrelated PE work.
  * each head's logits matmul writes its OWN psum bank (concurrent
    row-tiled matmuls into one bank are a fatal PSUM port conflict);
    col-tiled (partition-sliced) outputs legally share one.
  * conv3x3 = 9 shifted matmuls (W handled by a zero-padded x copy, H by
    shrunken row ranges + PSUM has_written semantics), paced one matmul per
    attention step to keep the PE dense (HAM clock stays warm).
"""

import sys

sys.path.insert(0, "/opt/trn_rl_repo")

import numpy as np

import concourse.bass as bass
import concourse.mybir as mybir
import concourse.tile as tile
from concourse import bacc
from concourse.bass_utils import run_bass_kernel_spmd
from concourse.tile_rust import add_dep_helper


def _order(after, before, reason):
    """Force `after` to execute after `before` (same-engine ordering edge)."""
    add_dep_helper(after.ins, before.ins, sync=False, reason=reason)


FP32 = mybir.dt.float32
BF16 = mybir.dt.bfloat16
AF = mybir.ActivationFunctionType
ALU = mybir.AluOpType

B, C, H, W = 8, 256, 32, 32
HW = H * W                      # 1024
NH, DK, DV = 8, 256, 256
DKH = DK // NH                  # 32
QSCALE = float(DKH) ** -0.5
P = 128
CT = C // P                     # 2 channel tiles of 128
MT = HW // P                    # 8 m-tiles (key/pixel axis)
N_CORES = 8


def _conv_mm_plan():
    """(ky,kx,it, row-range) matmuls for the 3x3 SAME conv per (cout-tile,
    half).  Center tap first: its start=True covers the full [16, 32] range,
    so the shrunken edge taps correctly overwrite-then-accumulate via the
    PSUM has_written bits."""
    taps = [(1, 1)] + [(ky, kx) for ky in range(3) for kx in range(3)
                       if (ky, kx) != (1, 1)]
    plan = []
    for cot in range(CT):
        for half in range(2):
            h0 = half * 16
            mms = []
            for ky, kx in taps:
                oy_lo = max(max(0, 1 - ky), h0)
                oy_hi = min(min(H, H + 1 - ky), h0 + 16)
                for it in range(CT):
                    mms.append(dict(
                        ky=ky, kx=kx, it=it,
                        oy=(oy_lo, oy_hi),
                        iy=(oy_lo + ky - 1, oy_hi + ky - 1),
                        first=(ky, kx, it) == (1, 1, 0),
                        last=(ky, kx) == taps[-1] and it == CT - 1,
                    ))
            plan.append(((cot, half), mms))
    return plan


def _finish_block(nc, pools, out_v, pa, pr, n0, sim_mode):
    """Normalize one block: denominators live in pa rows 32 and 96.  Two
    K=1 ones-matmuls (fp32r: full PE rate) replicate them across the 128
    output partitions of a spare psum tile, then DVE divides."""
    tmp_pool, psum_misc, ones64_bf = pools
    # DVE cannot move data across partitions: the den rows stay on their
    # native partitions (32 / 96) and the K=1 matmuls read them there.
    den = tmp_pool.tile([P, 512], BF16, tag="den", name="den")
    nc.vector.tensor_copy(out=den[DKH:DKH + 1, :], in_=pa[DKH:DKH + 1, :])
    nc.vector.tensor_copy(out=den[64 + DKH:64 + DKH + 1, :],
                          in_=pa[64 + DKH:64 + DKH + 1, :])
    denb = psum_misc.tile([P, 512], FP32, tag="mps", name="mps")
    for j in range(2):
        p0 = 64 * j + DKH
        nc.tensor.matmul(
            denb[64 * j:64 * (j + 1), :],
            lhsT=ones64_bf[p0:p0 + 1, :],
            rhs=den[p0:p0 + 1, :],
            start=True, stop=True,
            tile_position=(p0, 64 * j), skip_group_check=True)
    rec = tmp_pool.tile([P, 512], FP32, tag="rec", name="rec")
    nc.vector.reciprocal_approx_fast(rec[:], denb[:])
    att = tmp_pool.tile([P, 512], FP32, tag="att", name="att")
    for j in range(2):
        h = 2 * pr + j
        nc.vector.tensor_mul(att[64 * j:64 * j + DKH, :],
                             pa[64 * j:64 * j + DKH, :],
                             rec[64 * j:64 * j + DKH, :])
        nc.sync.dma_start(
            out=out_v[(h % 4) * DKH:(h % 4 + 1) * DKH, 1, h // 4,
                      n0:n0 + 512],
            in_=att[64 * j:64 * j + DKH, :])


def build_nc(interleave_conv=True, sim_mode=False):
    nc = bacc.Bacc(
        "TRN2",
        target_bir_lowering=False,
        debug=False,
        enable_asserts=False,
    )

    x_ext = nc.declare_dram_parameter("x", [C, H, W], FP32, isOutput=False)
    cw_ext = nc.declare_dram_parameter("conv_w", [3, 3, C, C], FP32, isOutput=False)
    cb_ext = nc.declare_dram_parameter("conv_b", [C], FP32, isOutput=False)
    aw_ext = nc.declare_dram_parameter("attn_w", [1, 1, C, 3 * C], FP32, isOutput=False)
    ab_ext = nc.declare_dram_parameter("attn_b", [3 * C], FP32, isOutput=False)
    out_ext = nc.declare_dram_parameter("out", [2 * C, H, W], FP32, isOutput=True)

    # output viewed as [p, group(conv/attn), ctile, hw]
    out_v = out_ext[:].rearrange("(g t p) h w -> p g t (h w)", g=2, t=CT, p=P)

    with tile.TileContext(nc) as tc:
        persist = tc.alloc_tile_pool(name="persist", bufs=1)
        expt_pool = tc.alloc_tile_pool(name="expt", bufs=3)
        tmp_pool = tc.alloc_tile_pool(name="tmp", bufs=2)
        psum_logit = tc.alloc_tile_pool(name="pslogit", bufs=2, space="PSUM")
        psum_attn = tc.alloc_tile_pool(name="psattn", bufs=2, space="PSUM")
        psum_misc = tc.alloc_tile_pool(name="psmisc", bufs=2, space="PSUM")

        # ---- ACT warm-up: pull the exp table load (~2.7us) into the DMA
        # window instead of the first real logits tile.
        ones64_bf = persist.tile([P, 64], BF16)
        nc.vector.memset(ones64_bf[:], 1.0)
        warm_in = persist.tile([P, 16], FP32)
        warm_out = persist.tile([P, 16], FP32)
        nc.vector.memset(warm_in[:], 0.0)
        nc.scalar.activation(warm_out[:], warm_in[:], AF.Exp)

        # ---- input DMAs: spread across the three DGE-capable queues
        # (sync / scalar / gpsimd) so transfers overlap; x first -- it gates
        # the kqv -> logits -> exp chain.
        x_f32 = persist.tile([P, CT, HW], FP32)
        nc.sync.dma_start(
            out=x_f32[:], in_=x_ext[:].rearrange("(t p) h w -> p t (h w)", p=P))

        aw_f32 = persist.tile([P, CT, 3 * C], FP32)
        nc.sync.dma_start(
            out=aw_f32[:],
            in_=aw_ext[:].rearrange("a b (it ip) o -> ip (a b it) o", ip=P))

        # conv_w: [ky kx it][256 cout] is byte-identical to [..][ot][op] but
        # gives 1 KiB descriptor runs; split across all three queues
        cw_f32 = persist.tile([P, 9, CT, 2 * P], FP32)
        cw_src = cw_ext[:].rearrange("ky kx (it ip) o -> ip (ky kx) it o", ip=P)
        nc.gpsimd.dma_start(out=cw_f32[:, 0:3], in_=cw_src[:, 0:3])
        nc.sync.dma_start(out=cw_f32[:, 3:6], in_=cw_src[:, 3:6])
        nc.gpsimd.dma_start(out=cw_f32[:, 6:9], in_=cw_src[:, 6:9])

        cb = persist.tile([P, CT], FP32)
        nc.gpsimd.dma_start(out=cb[:], in_=cb_ext[:].rearrange("(t p) -> p t", p=P))
        kb = persist.tile([P, CT], FP32)
        nc.gpsimd.dma_start(out=kb[:], in_=ab_ext[0:C].rearrange("(t p) -> p t", p=P))
        qb = persist.tile([P, CT], FP32)
        nc.gpsimd.dma_start(out=qb[:], in_=ab_ext[C:2 * C].rearrange("(t p) -> p t", p=P))
        # v bias replicated across partitions (it adds along vT's free axis)
        vb = persist.tile([P, C], FP32)
        vb_src = bass.AP(tensor=ab_ext.tensor if hasattr(ab_ext, "tensor")
                         else ab_ext[:].tensor,
                         offset=2 * C, ap=[[0, P], [1, C]])
        nc.gpsimd.dma_start(out=vb[:], in_=vb_src)

        qbs = persist.tile([P, CT], FP32)
        nc.vector.tensor_scalar_mul(qbs[:], qb[:], QSCALE)

        # ---- bf16 casts (DVE for the kqv-critical ones; conv weights split
        # DVE / ScalarE / GpSimd so conv matmuls can start early)
        x_bf = persist.tile([P, CT, HW], BF16)
        nc.vector.tensor_copy(out=x_bf[:], in_=x_f32[:])
        aw_bf = persist.tile([P, CT, 3 * C], BF16)
        nc.vector.tensor_copy(out=aw_bf[:], in_=aw_f32[:])
        cw_bf = persist.tile([P, 9, CT, 2 * P], BF16)
        cwf_flat = cw_f32[:].rearrange("p a b c -> p (a b c)")
        cwb_flat = cw_bf[:].rearrange("p a b c -> p (a b c)")
        wtot = 9 * CT * 2 * P
        c1, c2 = wtot * 4 // 9, wtot * 7 // 9
        nc.vector.tensor_copy(out=cwb_flat[:, :c1], in_=cwf_flat[:, :c1])
        nc.scalar.activation(cwb_flat[:, c1:c2], cwf_flat[:, c1:c2], AF.Copy)
        nc.gpsimd.tensor_copy(out=cwb_flat[:, c2:], in_=cwf_flat[:, c2:])

        # ---- kqv (k,q in [c, hw] layout) interleaved with vT = x^T @ Wv.
        # vT_ext[:, mt, h, 0:32] = vT for head h; [.., 32] = 1.0 -- the ones
        # column makes the attn matmul also produce the softmax denominator
        # (row 32 of each head's 33-row output strip).
        k_bf = persist.tile([P, CT, HW], BF16)
        q_bf = persist.tile([P, CT, HW], BF16)
        vT_ext = persist.tile([P, MT, NH, DKH + 1], BF16)
        nc.vector.memset(vT_ext[:, :, :, DKH:DKH + 1], 1.0)
        vb3 = vb[:].rearrange("p (h d) -> p h d", h=NH)

        def kqv_unit(u):
            ot, nh = u // 2, u % 2
            ps = psum_misc.tile([P, 512], FP32, tag="mps", name="mps")
            for it in range(CT):
                nc.tensor.matmul(
                    ps[:],
                    lhsT=aw_bf[:, it, ot * P:(ot + 1) * P],
                    rhs=x_bf[:, it, nh * 512:(nh + 1) * 512],
                    start=(it == 0), stop=(it == CT - 1))
            if ot < 2:
                nc.vector.tensor_scalar_add(
                    k_bf[:, ot, nh * 512:(nh + 1) * 512], ps[:],
                    kb[:, ot:ot + 1])
            else:
                # q = (psum + b) * s  ==  psum * s + (b * s)
                nc.vector.tensor_scalar(
                    q_bf[:, ot - 2, nh * 512:(nh + 1) * 512], ps[:],
                    QSCALE, qbs[:, ot - 2:ot - 1], ALU.mult, ALU.add)

        def vt_unit(mt):
            ps = psum_misc.tile([P, 512], FP32, tag="mps", name="mps")
            for it in range(CT):
                nc.tensor.matmul(
                    ps[:, :C],
                    lhsT=x_bf[:, it, mt * P:(mt + 1) * P],
                    rhs=aw_bf[:, it, 2 * C:3 * C],
                    start=(it == 0), stop=(it == CT - 1))
            nc.vector.tensor_add(
                vT_ext[:, mt, :, 0:DKH],
                ps[:, :C].rearrange("p (h d) -> p h d", h=NH), vb3[:])

        for u in range(8):
            kqv_unit(u)
            vt_unit(u)

        # ---- conv (paced into the attention pipeline) ---------------------
        WP = W + 2
        x_pad = persist.tile([P, CT, H, WP], BF16)
        nc.vector.memset(x_pad[:], 0.0)
        xv_bf = x_bf[:].rearrange("p t (h w) -> p t h w", h=H)
        nc.vector.tensor_copy(out=x_pad[:, :, :, 1:1 + W], in_=xv_bf)

        def emit_conv_mm(key, mm, ps):
            rhs = x_pad[:, mm["it"], mm["iy"][0]:mm["iy"][1],
                        mm["kx"]:mm["kx"] + W]
            h0 = key[1] * 16
            out = ps[:].rearrange("p a b -> p (a b)")[
                :, (mm["oy"][0] - h0) * W:(mm["oy"][1] - h0) * W]
            lhsT = cw_bf[:, mm["ky"] * 3 + mm["kx"], mm["it"],
                         key[0] * P:(key[0] + 1) * P]
            return nc.tensor.matmul(out, lhsT=lhsT, rhs=rhs,
                                    start=mm["first"], stop=mm["last"])

        def conv_gen():
            for key, mms in _conv_mm_plan():
                cot, half = key
                ps = psum_misc.tile([P, 16, W], FP32, tag="mps", name="mps")
                for mm in mms:
                    yield emit_conv_mm(key, mm, ps)
                cout = tmp_pool.tile([P, 16, W], FP32, tag="cout", name="cout")
                nc.vector.tensor_scalar_add(cout[:], ps[:], cb[:, cot:cot + 1])
                nc.sync.dma_start(
                    out=out_v[:, 0, cot, half * 512:(half + 1) * 512],
                    in_=cout[:].rearrange("p a b -> p (a b)"))
                yield None

        conv_iter = conv_gen() if interleave_conv else None

        def pump_conv(n, anchor=None):
            if conv_iter is None:
                return
            for _ in range(n):
                try:
                    mm = next(conv_iter)
                except StopIteration:
                    break
                if mm is not None and anchor is not None:
                    # pace conv with the attention stream (the scheduler
                    # would otherwise front-load it all, starving the late
                    # region and re-throttling the PE clock)
                    _order(mm, anchor, "pace conv")

        # ---- attention: flat 64-step software pipeline --------------------
        seq = [(pr, nt, mt) for pr in range(4) for nt in range(2)
               for mt in range(MT)]
        pa_by_block = {}
        exs = {}
        for s in range(len(seq) + 2):
            if s < len(seq):
                pr, nt, mt = seq[s]
                hgt = pr // 2              # k/q channel tile of this pair
                pbase = 64 * (pr % 2)      # its partition base inside that
                n0 = nt * 512
                pl = psum_logit.tile([P, 2, 512], FP32, tag="pl", name="pl")
                anchor = None
                for j in range(2):
                    p0 = pbase + 32 * j
                    anchor = nc.tensor.matmul(
                        pl[:, j, :],
                        lhsT=k_bf[p0:p0 + 32, hgt, mt * P:(mt + 1) * P],
                        rhs=q_bf[p0:p0 + 32, hgt, n0:n0 + 512],
                        start=True, stop=True,
                        tile_position=(p0, 0))
                ex = expt_pool.tile([P, 2, 512], BF16, tag="ex", name="ex")
                nc.scalar.activation(ex[:], pl[:], AF.Exp)
                exs[s] = ex
                pump_conv(1 + (s % 5 == 0), anchor)
            if s >= 2:
                pr, nt, mt = seq[s - 2]
                n0 = nt * 512
                if mt == 0:
                    pa_by_block[(pr, nt)] = psum_attn.tile(
                        [P, 512], FP32, tag="pa", name="pa")
                pa = pa_by_block[(pr, nt)]
                ex = exs.pop(s - 2)
                # attn@v + denominator in one matmul per head: lhsT is
                # [vT_h | ones] (33 cols) at 64-wide col groups -> rows
                # 0..31 attn, row 32 denom, two heads fully concurrent.
                for j in range(2):
                    h = 2 * pr + j
                    nc.tensor.matmul(
                        pa[64 * j:64 * j + DKH + 1, :],
                        lhsT=vT_ext[:, mt, h, :],
                        rhs=ex[:, j, :],
                        start=(mt == 0), stop=(mt == MT - 1),
                        tile_position=(0, 64 * j), skip_group_check=True)
                if mt == MT - 1:
                    _finish_block(nc, (tmp_pool, psum_misc, ones64_bf), out_v, pa, pr, n0, sim_mode)
                    del pa_by_block[(pr, nt)]

        # drain leftover conv work
        pump_conv(10000)
        if conv_iter is None:
            for _ in conv_gen():
                pass

        for pool in (psum_misc, psum_attn, psum_logit, tmp_pool,
                     expt_pool, persist):
            pool.release()

    nc.compile()
    return nc


_CACHED_NC = None


def _get_nc():
    global _CACHED_NC
    if _CACHED_NC is None:
        _CACHED_NC = build_nc()
    return _CACHED_NC


def run_spmd(nc, inputs, trace=False, **kw):
    in_maps = []
    for i in range(N_CORES):
        in_maps.append({
            "x": np.ascontiguousarray(inputs["x"][i], dtype=np.float32),
            "conv_w": np.ascontiguousarray(inputs["conv_w"], dtype=np.float32),
            "conv_b": np.ascontiguousarray(inputs["conv_b"], dtype=np.float32),
            "attn_w": np.ascontiguousarray(inputs["attn_w"], dtype=np.float32),
            "attn_b": np.ascontiguousarray(inputs["attn_b"], dtype=np.float32),
        })
    res = run_bass_kernel_spmd(nc, in_maps, core_ids=list(range(N_CORES)),
                               trace=trace, **kw)
    out = np.stack([np.asarray(res.results[i]["out"]) for i in range(N_CORES)],
                   axis=0)
    return out, res


def kernel(**inputs):
    nc = _get_nc()
    out, _ = run_spmd(nc, inputs, trace=False)
    return out.astype(np.float32)


if __name__ == "__main__":
    nc = build_nc()
    print("build + compile OK")
